# revision 7
# baseline (speedup 1.0000x reference)
"""Trainium2 Bass kernel for nn_Decoder_40338332844507.

Computes logits = einsum('btc,wpc->bptw', q, W) + b.T[None,:,None,:]
with q [32, 2048, 256] f32, W [49, 32, 256] f32, b [49, 32] f32,
output [32, 32, 2048, 49] f32.

Strategy: data-parallel over batch across 8 NeuronCores (4 batches per
core). All device-side math runs in fp16 (inputs rounded on host,
output stored as fp16 and upcast on host) — end-to-end rel err ~4e-4
vs the 2e-2 gate. This halves the store stream (51.4 -> 25.7 MB/core),
which was the baseline's critical path, and moves the kernel onto the
compute/memory ridge.

Per 128-token tile (t = tp*16 + tl), one [128, 1568] PSUM tile holds
all P*W outputs; the stationary q tile [c,128] is loaded once per
c-half and reused by 4 matmuls over the full 1568 moving columns (a
BIR post-pass drops the redundant Ldweights bass emits per matmul,
~22us/core of PE time). The f32->fp16 downcast eviction is split
between the DVE (p 0:16) and Activation (p 16:32) engines (GPSIMD
cannot read PSUM); the per-(p,w) bias is added on the host during the
f32 upcast, fused into one numpy pass. The last batch is processed in
p-strips with separate tl sweeps so the final stores are small (short
tail after the last matmul). Token-interleaved stores give 16*49*2-byte
contiguous DRAM runs per descriptor line.
"""

import json
import sys
import numpy as np
from contextlib import ExitStack

if "/opt/trn_rl_repo" not in sys.path:
    sys.path.insert(0, "/opt/trn_rl_repo")

import concourse.bass as bass
import concourse.tile as tile
from concourse import mybir
from concourse.bass_utils import run_bass_kernel_spmd

B, T, C = 32, 2048, 256
P, WW = 32, 49
N = P * WW  # 1568
N_CORES = 8
B_LOC = B // N_CORES  # 4 batches per core
TL = 16  # token interleave: t = tp*16 + tl -> store runs of 16*49*2 B

USE_LD = True  # explicit ldweights + no-load matmuls (stationary reuse)


def _patch_split_sync_waits():
    """The walrus build on this image accepts at most ONE sync-wait per
    instruction ("Too many sync wait commands" otherwise). Tile emits
    instructions with several waits. Post-process the serialized BIR:
    hoist all but the last wait of each instruction onto 1-wait NoOps
    inserted immediately before it on the same engine (engines execute
    their instruction stream in order, so the semantics are identical)."""
    if getattr(bass.Bass, "_split_waits_patched", False):
        return
    orig = bass.Bass.to_json_bytes

    def to_json_bytes(self):
        m = json.loads(orig(self))
        # --- pass 1: drop redundant Ldweights -------------------------
        # bass serialization splits every Matmult into Ldweights +
        # Matmult(ldweights=False). Consecutive matmuls that reuse the
        # same stationary tile re-load it for nothing (~128 PE cycles
        # each). Drop an Ldweights when the previous one on the same
        # engine had an identical weights AP and only Matmult/NoOp
        # instructions executed in between; keep its sync_info on a NoOp.
        for f in m.get("functions", []):
            for bb in f.get("blocks", []):
                out = []
                last_sig = None
                for inst in bb.get("instructions", []):
                    if inst["engine"] != "PE":
                        out.append(inst)
                        continue
                    op = inst["opcode"]
                    if op == "Ldweights":
                        sig = json.dumps(
                            [
                                inst.get("ins"),
                                inst.get("is_transpose"),
                                inst.get("perf_mode"),
                                inst.get("tile_position"),
                                inst.get("tile_size"),
                            ],
                            sort_keys=True,
                        )
                        if sig == last_sig:
                            si = inst.get("sync_info")
                            if si and (si.get("on_wait") or si.get("on_update")):
                                nop = {
                                    "engine": "PE",
                                    "ins": [],
                                    "outs": [],
                                    "name": inst["name"] + "w",
                                    "opcode": "NoOp",
                                    "sync_info": si,
                                }
                                if inst.get("debug") is not None:
                                    nop["debug"] = inst["debug"]
                                out.append(nop)
                            continue  # drop the redundant load
                        last_sig = sig
                    elif op not in ("Matmult", "NoOp"):
                        last_sig = None
                    out.append(inst)
                bb["instructions"] = out
        # --- pass 2: split multi-wait sync_info onto NoOps ------------
        ctr = 0
        for f in m.get("functions", []):
            for bb in f.get("blocks", []):
                out = []
                for inst in bb.get("instructions", []):
                    si = inst.get("sync_info")
                    if si:
                        waits = si.get("on_wait") or []
                        if len(waits) > 1:
                            for wt in waits[:-1]:
                                ctr += 1
                                nop = {
                                    "engine": inst["engine"],
                                    "ins": [],
                                    "outs": [],
                                    "name": f"I-npw{ctr}",
                                    "opcode": "NoOp",
                                    "sync_info": {"on_wait": [wt], "on_update": []},
                                }
                                if inst.get("debug") is not None:
                                    nop["debug"] = inst["debug"]
                                out.append(nop)
                            si["on_wait"] = waits[-1:]
                    out.append(inst)
                bb["instructions"] = out
        return json.dumps(m).encode()

    bass.Bass.to_json_bytes = to_json_bytes
    bass.Bass._split_waits_patched = True


def _mm_noload(eng, out, lhsT, rhs, start, stop):
    """InstMatmult with ldweights=False: reuses the stationary already
    in the PE array (loaded by the preceding self-loading matmul with
    the same lhsT). lhsT is still passed as an input so Tile tracks the
    dependency, but walrus skips the redundant LDWEIGHTS."""
    ifmap_ap = eng.lower_ap(rhs.opt({0}), opt=False)
    weights_ap = eng.lower_ap(lhsT.opt({0}), opt=False, for_matmul_weights=True)
    out_ap = eng.lower_ap(out)
    return eng.add_instruction(
        mybir.InstMatmult(
            name=eng.bass.get_next_instruction_name(),
            replication_resolution=0,
            replication_shift_amnt=0,
            replication_num_rows=0,
            start_tensor_calc=start,
            stop_tensor_calc=stop,
            ldweights=False,
            ins=[ifmap_ap, weights_ap],
            outs=[out_ap],
            perf_mode=None,
            is_transpose=None,
            ifmap_quant_offset=None,
            weights_quant_offset=None,
            bass_skip_group_check=False,
            tile_position=(0, 0),
            tile_size=(128, 128),
        )
    )


def build_bass():
    _patch_split_sync_waits()
    nc = bass.Bass("TRN2", target_bir_lowering=False, debug=False)
    f32 = mybir.dt.float32
    fp16 = mybir.dt.float16

    qt = nc.dram_tensor("qt", [B_LOC, C, T], fp16, kind="ExternalInput")
    wr = nc.dram_tensor("wr", [C, N], fp16, kind="ExternalInput")
    o = nc.dram_tensor("o", [B_LOC, P, T, WW], fp16, kind="ExternalOutput")

    with tile.TileContext(nc) as tc:
        with ExitStack() as ctx:
            consts = ctx.enter_context(tc.tile_pool(name="consts", bufs=1))
            qpool = ctx.enter_context(tc.tile_pool(name="qpool", bufs=2))
            opool = ctx.enter_context(tc.tile_pool(name="opool", bufs=2))
            psum = ctx.enter_context(tc.tile_pool(name="psum", bufs=2, space="PSUM"))

            wr_sb = [
                consts.tile([128, N], fp16, tag=f"wr{k}", name=f"wr{k}")
                for k in range(2)
            ]
            nc.sync.dma_start(wr_sb[0][:], wr.ap()[0:128, :])
            nc.scalar.dma_start(wr_sb[1][:], wr.ap()[128:256, :])

            def load_q(b, eng):
                q_sb = [
                    qpool.tile([128, T], fp16, tag=f"q{k}", name=f"q{k}_{b}")
                    for k in range(2)
                ]
                eng.dma_start(q_sb[0][:], qt.ap()[b, 0:128, :])
                eng.dma_start(q_sb[1][:], qt.ap()[b, 128:256, :])
                # t split as (tp, tl); lhsT slices are [c, tp] (stride TL)
                return [q_sb[k][:].rearrange("c (p l) -> c l p", l=TL) for k in range(2)]

            def compute_tl(pt, q_v, tl, nbase, nw):
                """Accumulate pt[:, 0:nw] = q_tile.T @ wr[:, nbase:nbase+nw]."""
                for k in range(2):
                    for n0 in range(0, nw, 512):
                        n1 = min(n0 + 512, nw)
                        if USE_LD and n0 > 0:
                            _mm_noload(
                                nc.tensor,
                                pt[:, n0:n1],
                                q_v[k][:, tl, :],
                                wr_sb[k][:, nbase + n0 : nbase + n1],
                                start=(k == 0),
                                stop=(k == 1),
                            )
                        else:
                            nc.tensor.matmul(
                                pt[:, n0:n1],
                                q_v[k][:, tl, :],
                                wr_sb[k][:, nbase + n0 : nbase + n1],
                                start=(k == 0),
                                stop=(k == 1),
                            )

            def evict(eng, oh, tl, pt, psum_off, np_):
                """PSUM f32 -> SBUF fp16 downcast copy (bias is added on
                the host). eng is nc.vector (DVE) or nc.scalar (Act)."""
                pv = pt[:, psum_off : psum_off + np_ * WW].rearrange(
                    "t (p w) -> t p w", w=WW
                )
                dst = oh[:, :, bass.ds(tl * WW, WW)]
                if eng is nc.scalar:
                    eng.copy(dst, pv[:])
                else:
                    eng.tensor_copy(dst, pv[:])

            def store(eng, oh, b, p0, np_):
                dst = (
                    o.ap()[b, p0 : p0 + np_, :, :]
                    .rearrange("p (t l) w -> t p (l w)", l=TL)
                )
                eng.dma_start(dst, oh[:, :, :])

            # ---- batches 0..2: full-width tiles, both p-halves per sweep ----
            q_v = load_q(0, nc.gpsimd)
            for b in range(3):
                oh = [
                    opool.tile([128, 16, TL * WW], fp16, tag=f"oh{h}", name=f"oh{h}_{b}")
                    for h in range(2)
                ]
                for tl in range(TL):
                    pt = psum.tile([128, 2048], f32, tag="pt", name=f"pt_{b}_{tl}")
                    compute_tl(pt, q_v, tl, 0, N)
                    evict(nc.vector, oh[0], tl, pt, 0, 16)
                    evict(nc.scalar, oh[1], tl, pt, 784, 16)
                    if tl == 0:
                        # prefetch next batch's q via the Pool SWDGE queue
                        q_v_next = load_q(b + 1, nc.gpsimd)
                store(nc.sync, oh[0], b, 0, 16)
                store(nc.scalar, oh[1], b, 16, 16)
                q_v = q_v_next

            # ---- batch 3: h0 full sweep, then h1 as two p-strips so the
            # final stores are small (short tail) ----
            oh0 = opool.tile([128, 16, TL * WW], fp16, tag="oh0", name="oh0_3")
            for tl in range(TL):
                pt = psum.tile([128, 2048], f32, tag="pt", name=f"pt_3h0_{tl}")
                compute_tl(pt, q_v, tl, 0, 784)
                evict(nc.vector if tl % 2 == 0 else nc.scalar, oh0, tl, pt, 0, 16)
            store(nc.sync, oh0, 3, 0, 16)
            for s in range(2):
                nbase = 784 + s * 392
                ohs = opool.tile([128, 8, TL * WW], fp16, tag="ohS", name=f"ohS_{s}")
                for tl in range(TL):
                    pt = psum.tile([128, 2048], f32, tag="pt", name=f"pt_3s{s}_{tl}")
                    compute_tl(pt, q_v, tl, nbase, 392)
                    evict(
                        nc.vector if tl % 2 == 0 else nc.scalar,
                        ohs, tl, pt, 0, 8,
                    )
                store(nc.scalar if s == 0 else nc.sync, ohs, 3, 16 + 8 * s, 8)
    return nc


_NC_CACHE = None


def _get_nc():
    global _NC_CACHE
    if _NC_CACHE is None:
        _NC_CACHE = build_bass()
    return _NC_CACHE


def prep_inputs(q, W, b):
    """Host-side layout prep: weight packing + activation transpose +
    fp16 cast."""
    qt = np.ascontiguousarray(
        np.asarray(q, dtype=np.float32).transpose(0, 2, 1)
    ).astype(np.float16)  # [B, C, T]
    wr = np.ascontiguousarray(
        np.asarray(W, dtype=np.float32).transpose(2, 1, 0).reshape(C, N)
    ).astype(np.float16)
    return qt, wr


def assemble_output(core_outs, b):
    """Concatenate per-core fp16 device outputs, upcast to f32 and add
    the bias (b is the [W, P] reference bias) in one fused pass."""
    dev = np.concatenate(core_outs, axis=0)  # [B, P, T, W] fp16
    bias = np.asarray(b, dtype=np.float32).T[None, :, None, :]  # [1,P,1,W]
    return np.add(dev, bias, dtype=np.float32)


def kernel(q, W, b):
    qt, wr = prep_inputs(q, W, b)
    nc = _get_nc()
    in_maps = [
        {
            "qt": qt[c * B_LOC : (c + 1) * B_LOC],
            "wr": wr,
        }
        for c in range(N_CORES)
    ]
    res = run_bass_kernel_spmd(nc, in_maps, core_ids=list(range(N_CORES)))
    return assemble_output(
        [res.results[c]["o"] for c in range(N_CORES)], b
    )


# revision 8
# speedup vs baseline: 1.2481x; 1.2481x over previous
"""Trainium2 Bass kernel for nn_Decoder_40338332844507.

Computes logits = einsum('btc,wpc->bptw', q, W) + b.T[None,:,None,:]
with q [32, 2048, 256] f32, W [49, 32, 256] f32, b [49, 32] f32,
output [32, 32, 2048, 49] f32.

Strategy: data-parallel over batch across 8 NeuronCores (4 batches per
core). All device-side math runs in fp16 (inputs rounded on host,
output stored as fp16 and upcast on host) — end-to-end rel err ~4e-4
vs the 2e-2 gate. This halves the store stream (51.4 -> 25.7 MB/core),
which was the baseline's critical path, and moves the kernel onto the
compute/memory ridge.

Per 128-token tile (t = tp*16 + tl), one [128, 1568] PSUM tile holds
all P*W outputs; the stationary q tile [c,128] is loaded once per
c-half and reused by 4 matmuls over the full 1568 moving columns (a
BIR post-pass drops the redundant Ldweights bass emits per matmul,
~22us/core of PE time). The f32->fp16 downcast eviction is split
between the DVE (p 0:16) and Activation (p 16:32) engines (GPSIMD
cannot read PSUM); the per-(p,w) bias is added on the host during the
f32 upcast, fused into one numpy pass. The last batch is processed in
p-strips with separate tl sweeps so the final stores are small (short
tail after the last matmul). Token-interleaved stores give 16*49*2-byte
contiguous DRAM runs per descriptor line.
"""

import json
import sys
import numpy as np
from contextlib import ExitStack

if "/opt/trn_rl_repo" not in sys.path:
    sys.path.insert(0, "/opt/trn_rl_repo")

import concourse.bass as bass
import concourse.tile as tile
from concourse import mybir
from concourse.bass_utils import run_bass_kernel_spmd

B, T, C = 32, 2048, 256
P, WW = 32, 49
N = P * WW  # 1568
N_CORES = 8
B_LOC = B // N_CORES  # 4 batches per core
TL = 16  # token interleave: t = tp*16 + tl -> store runs of 16*49*2 B

USE_LD = True  # explicit ldweights + no-load matmuls (stationary reuse)


def _patch_split_sync_waits():
    """The walrus build on this image accepts at most ONE sync-wait per
    instruction ("Too many sync wait commands" otherwise). Tile emits
    instructions with several waits. Post-process the serialized BIR:
    hoist all but the last wait of each instruction onto 1-wait NoOps
    inserted immediately before it on the same engine (engines execute
    their instruction stream in order, so the semantics are identical)."""
    if getattr(bass.Bass, "_split_waits_patched", False):
        return
    orig = bass.Bass.to_json_bytes

    def to_json_bytes(self):
        m = json.loads(orig(self))
        # --- pass 1: drop redundant Ldweights -------------------------
        # bass serialization splits every Matmult into Ldweights +
        # Matmult(ldweights=False). Consecutive matmuls that reuse the
        # same stationary tile re-load it for nothing (~128 PE cycles
        # each). Drop an Ldweights when the previous one on the same
        # engine had an identical weights AP and only Matmult/NoOp
        # instructions executed in between; keep its sync_info on a NoOp.
        for f in m.get("functions", []):
            for bb in f.get("blocks", []):
                out = []
                last_sig = None
                for inst in bb.get("instructions", []):
                    if inst["engine"] != "PE":
                        out.append(inst)
                        continue
                    op = inst["opcode"]
                    if op == "Ldweights":
                        sig = json.dumps(
                            [
                                inst.get("ins"),
                                inst.get("is_transpose"),
                                inst.get("perf_mode"),
                                inst.get("tile_position"),
                                inst.get("tile_size"),
                            ],
                            sort_keys=True,
                        )
                        if sig == last_sig:
                            si = inst.get("sync_info")
                            if si and (si.get("on_wait") or si.get("on_update")):
                                nop = {
                                    "engine": "PE",
                                    "ins": [],
                                    "outs": [],
                                    "name": inst["name"] + "w",
                                    "opcode": "NoOp",
                                    "sync_info": si,
                                }
                                if inst.get("debug") is not None:
                                    nop["debug"] = inst["debug"]
                                out.append(nop)
                            continue  # drop the redundant load
                        last_sig = sig
                    elif op not in ("Matmult", "NoOp"):
                        last_sig = None
                    out.append(inst)
                bb["instructions"] = out
        # --- pass 2: split multi-wait sync_info onto NoOps ------------
        ctr = 0
        for f in m.get("functions", []):
            for bb in f.get("blocks", []):
                out = []
                for inst in bb.get("instructions", []):
                    si = inst.get("sync_info")
                    if si:
                        waits = si.get("on_wait") or []
                        if len(waits) > 1:
                            for wt in waits[:-1]:
                                ctr += 1
                                nop = {
                                    "engine": inst["engine"],
                                    "ins": [],
                                    "outs": [],
                                    "name": f"I-npw{ctr}",
                                    "opcode": "NoOp",
                                    "sync_info": {"on_wait": [wt], "on_update": []},
                                }
                                if inst.get("debug") is not None:
                                    nop["debug"] = inst["debug"]
                                out.append(nop)
                            si["on_wait"] = waits[-1:]
                    out.append(inst)
                bb["instructions"] = out
        return json.dumps(m).encode()

    bass.Bass.to_json_bytes = to_json_bytes
    bass.Bass._split_waits_patched = True


def _mm_noload(eng, out, lhsT, rhs, start, stop):
    """InstMatmult with ldweights=False: reuses the stationary already
    in the PE array (loaded by the preceding self-loading matmul with
    the same lhsT). lhsT is still passed as an input so Tile tracks the
    dependency, but walrus skips the redundant LDWEIGHTS."""
    ifmap_ap = eng.lower_ap(rhs.opt({0}), opt=False)
    weights_ap = eng.lower_ap(lhsT.opt({0}), opt=False, for_matmul_weights=True)
    out_ap = eng.lower_ap(out)
    return eng.add_instruction(
        mybir.InstMatmult(
            name=eng.bass.get_next_instruction_name(),
            replication_resolution=0,
            replication_shift_amnt=0,
            replication_num_rows=0,
            start_tensor_calc=start,
            stop_tensor_calc=stop,
            ldweights=False,
            ins=[ifmap_ap, weights_ap],
            outs=[out_ap],
            perf_mode=None,
            is_transpose=None,
            ifmap_quant_offset=None,
            weights_quant_offset=None,
            bass_skip_group_check=False,
            tile_position=(0, 0),
            tile_size=(128, 128),
        )
    )


def build_bass():
    _patch_split_sync_waits()
    nc = bass.Bass("TRN2", target_bir_lowering=False, debug=False)
    f32 = mybir.dt.float32
    fp16 = mybir.dt.float16

    qt = nc.dram_tensor("qt", [B_LOC, C, T], fp16, kind="ExternalInput")
    wr = nc.dram_tensor("wr", [C, N], fp16, kind="ExternalInput")
    o = nc.dram_tensor("o", [B_LOC, P, T, WW], fp16, kind="ExternalOutput")

    with tile.TileContext(nc) as tc:
        with ExitStack() as ctx:
            consts = ctx.enter_context(tc.tile_pool(name="consts", bufs=1))
            qpool = ctx.enter_context(tc.tile_pool(name="qpool", bufs=2))
            opool = ctx.enter_context(tc.tile_pool(name="opool", bufs=2))
            psum = ctx.enter_context(tc.tile_pool(name="psum", bufs=4, space="PSUM"))

            wr_sb = [
                consts.tile([128, N], fp16, tag=f"wr{k}", name=f"wr{k}")
                for k in range(2)
            ]
            nc.sync.dma_start(wr_sb[0][:], wr.ap()[0:128, :])
            nc.scalar.dma_start(wr_sb[1][:], wr.ap()[128:256, :])

            def load_q(b, eng):
                q_sb = [
                    qpool.tile([128, T], fp16, tag=f"q{k}", name=f"q{k}_{b}")
                    for k in range(2)
                ]
                eng.dma_start(q_sb[0][:], qt.ap()[b, 0:128, :])
                eng.dma_start(q_sb[1][:], qt.ap()[b, 128:256, :])
                # t split as (tp, tl); lhsT slices are [c, tp] (stride TL)
                return [q_sb[k][:].rearrange("c (p l) -> c l p", l=TL) for k in range(2)]

            def compute_tl(pieces, q_v, tl):
                """pieces: list of (pt, nbase, nw). For each contraction
                half k, the stationary q tile is loaded once (self-loading
                first matmul) and reused by every subsequent chunk across
                all pieces (ldweights=False)."""
                for k in range(2):
                    first = True
                    for pt, nbase, nw in pieces:
                        for n0 in range(0, nw, 512):
                            n1 = min(n0 + 512, nw)
                            if USE_LD and not first:
                                _mm_noload(
                                    nc.tensor,
                                    pt[:, n0:n1],
                                    q_v[k][:, tl, :],
                                    wr_sb[k][:, nbase + n0 : nbase + n1],
                                    start=(k == 0),
                                    stop=(k == 1),
                                )
                            else:
                                nc.tensor.matmul(
                                    pt[:, n0:n1],
                                    q_v[k][:, tl, :],
                                    wr_sb[k][:, nbase + n0 : nbase + n1],
                                    start=(k == 0),
                                    stop=(k == 1),
                                )
                            first = False

            def evict(eng, oh, tl, pt, np_):
                """PSUM f32 -> SBUF fp16 downcast copy (bias is added on
                the host). eng is nc.vector (DVE) or nc.scalar (Act)."""
                pv = pt[:, 0 : np_ * WW].rearrange(
                    "t (p w) -> t p w", w=WW
                )
                dst = oh[:, :, bass.ds(tl * WW, WW)]
                if eng is nc.scalar:
                    eng.copy(dst, pv[:])
                else:
                    eng.tensor_copy(dst, pv[:])

            def store(eng, oh, b, p0, np_):
                dst = (
                    o.ap()[b, p0 : p0 + np_, :, :]
                    .rearrange("p (t l) w -> t p (l w)", l=TL)
                )
                eng.dma_start(dst, oh[:, :, :])

            # ---- batches 0..2: full-width tiles, both p-halves per sweep ----
            # first q load on the fast sync HWDGE queue so the PE starts early
            q_v = load_q(0, nc.sync)
            for b in range(3):
                oh = [
                    opool.tile([128, 16, TL * WW], fp16, tag=f"oh{h}", name=f"oh{h}_{b}")
                    for h in range(2)
                ]
                for tl in range(TL):
                    # two 2-bank PSUM tiles per tl (ring of 4) so evictions
                    # have 2 tl of slack and never stall the PE
                    ptA = psum.tile([128, 1024], f32, tag="pt", name=f"ptA_{b}_{tl}")
                    ptB = psum.tile([128, 1024], f32, tag="pt", name=f"ptB_{b}_{tl}")
                    compute_tl([(ptA, 0, 784), (ptB, 784, 784)], q_v, tl)
                    evict(nc.vector, oh[0], tl, ptA, 16)
                    evict(nc.scalar, oh[1], tl, ptB, 16)
                    if tl == 0:
                        # prefetch next batch's q via the Pool SWDGE queue
                        q_v_next = load_q(b + 1, nc.gpsimd)
                store(nc.sync, oh[0], b, 0, 16)
                store(nc.scalar, oh[1], b, 16, 16)
                q_v = q_v_next

            # ---- batch 3: h0 full sweep, then h1 as two p-strips so the
            # final stores are small (short tail) ----
            oh0 = opool.tile([128, 16, TL * WW], fp16, tag="oh0", name="oh0_3")
            for tl in range(TL):
                pt = psum.tile([128, 1024], f32, tag="pt", name=f"pt_3h0_{tl}")
                compute_tl([(pt, 0, 784)], q_v, tl)
                evict(nc.vector if tl % 2 == 0 else nc.scalar, oh0, tl, pt, 16)
            store(nc.sync, oh0, 3, 0, 16)
            for s in range(2):
                nbase = 784 + s * 392
                ohs = opool.tile([128, 8, TL * WW], fp16, tag="ohS", name=f"ohS_{s}")
                for tl in range(TL):
                    pt = psum.tile([128, 1024], f32, tag="pt", name=f"pt_3s{s}_{tl}")
                    compute_tl([(pt, nbase, 392)], q_v, tl)
                    evict(
                        nc.vector if tl % 2 == 0 else nc.scalar,
                        ohs, tl, pt, 8,
                    )
                store(nc.scalar if s == 0 else nc.sync, ohs, 3, 16 + 8 * s, 8)
    return nc


_NC_CACHE = None


def _get_nc():
    global _NC_CACHE
    if _NC_CACHE is None:
        _NC_CACHE = build_bass()
    return _NC_CACHE


def prep_inputs(q, W, b):
    """Host-side layout prep: weight packing + activation transpose +
    fp16 cast."""
    qt = np.ascontiguousarray(
        np.asarray(q, dtype=np.float32).transpose(0, 2, 1)
    ).astype(np.float16)  # [B, C, T]
    wr = np.ascontiguousarray(
        np.asarray(W, dtype=np.float32).transpose(2, 1, 0).reshape(C, N)
    ).astype(np.float16)
    return qt, wr


def assemble_output(core_outs, b):
    """Concatenate per-core fp16 device outputs, upcast to f32 and add
    the bias (b is the [W, P] reference bias) in one fused pass."""
    dev = np.concatenate(core_outs, axis=0)  # [B, P, T, W] fp16
    bias = np.asarray(b, dtype=np.float32).T[None, :, None, :]  # [1,P,1,W]
    return np.add(dev, bias, dtype=np.float32)


def kernel(q, W, b):
    qt, wr = prep_inputs(q, W, b)
    nc = _get_nc()
    in_maps = [
        {
            "qt": qt[c * B_LOC : (c + 1) * B_LOC],
            "wr": wr,
        }
        for c in range(N_CORES)
    ]
    res = run_bass_kernel_spmd(nc, in_maps, core_ids=list(range(N_CORES)))
    return assemble_output(
        [res.results[c]["o"] for c in range(N_CORES)], b
    )


# revision 9
# speedup vs baseline: 1.3368x; 1.0710x over previous
"""Trainium2 Bass kernel for nn_Decoder_40338332844507.

Computes logits = einsum('btc,wpc->bptw', q, W) + b.T[None,:,None,:]
with q [32, 2048, 256] f32, W [49, 32, 256] f32, b [49, 32] f32,
output [32, 32, 2048, 49] f32.

Strategy: data-parallel over batch across 8 NeuronCores (4 batches per
core). All device-side math runs in fp16 (inputs rounded on host,
output stored as fp16 and upcast on host) — end-to-end rel err ~4e-4
vs the 2e-2 gate. This halves the store stream (51.4 -> 25.7 MB/core),
which was the baseline's critical path, and moves the kernel onto the
compute/memory ridge.

Per 128-token tile (t = tp*16 + tl), one [128, 1568] PSUM tile holds
all P*W outputs; the stationary q tile [c,128] is loaded once per
c-half and reused by 4 matmuls over the full 1568 moving columns (a
BIR post-pass drops the redundant Ldweights bass emits per matmul,
~22us/core of PE time). The f32->fp16 downcast eviction is split
between the DVE (p 0:16) and Activation (p 16:32) engines (GPSIMD
cannot read PSUM); the per-(p,w) bias is added on the host during the
f32 upcast, fused into one numpy pass. The last batch is processed in
p-strips with separate tl sweeps so the final stores are small (short
tail after the last matmul). Token-interleaved stores give 16*49*2-byte
contiguous DRAM runs per descriptor line.
"""

import json
import sys
import numpy as np
from contextlib import ExitStack

if "/opt/trn_rl_repo" not in sys.path:
    sys.path.insert(0, "/opt/trn_rl_repo")

import concourse.bass as bass
import concourse.tile as tile
from concourse import mybir
from concourse.bass_utils import run_bass_kernel_spmd

B, T, C = 32, 2048, 256
P, WW = 32, 49
N = P * WW  # 1568
N_CORES = 8
B_LOC = B // N_CORES  # 4 batches per core
TL = 16  # token interleave: t = tp*16 + tl -> store runs of 16*49*2 B

USE_LD = True  # explicit ldweights + no-load matmuls (stationary reuse)


def _patch_split_sync_waits():
    """The walrus build on this image accepts at most ONE sync-wait per
    instruction ("Too many sync wait commands" otherwise). Tile emits
    instructions with several waits. Post-process the serialized BIR:
    hoist all but the last wait of each instruction onto 1-wait NoOps
    inserted immediately before it on the same engine (engines execute
    their instruction stream in order, so the semantics are identical)."""
    if getattr(bass.Bass, "_split_waits_patched", False):
        return
    orig = bass.Bass.to_json_bytes

    def to_json_bytes(self):
        m = json.loads(orig(self))
        # --- pass 1: drop redundant Ldweights -------------------------
        # bass serialization splits every Matmult into Ldweights +
        # Matmult(ldweights=False). Consecutive matmuls that reuse the
        # same stationary tile re-load it for nothing (~128 PE cycles
        # each). Drop an Ldweights when the previous one on the same
        # engine had an identical weights AP and only Matmult/NoOp
        # instructions executed in between; keep its sync_info on a NoOp.
        for f in m.get("functions", []):
            for bb in f.get("blocks", []):
                out = []
                last_sig = None
                for inst in bb.get("instructions", []):
                    if inst["engine"] != "PE":
                        out.append(inst)
                        continue
                    op = inst["opcode"]
                    if op == "Ldweights":
                        sig = json.dumps(
                            [
                                inst.get("ins"),
                                inst.get("is_transpose"),
                                inst.get("perf_mode"),
                                inst.get("tile_position"),
                                inst.get("tile_size"),
                            ],
                            sort_keys=True,
                        )
                        if sig == last_sig:
                            si = inst.get("sync_info")
                            if si and (si.get("on_wait") or si.get("on_update")):
                                nop = {
                                    "engine": "PE",
                                    "ins": [],
                                    "outs": [],
                                    "name": inst["name"] + "w",
                                    "opcode": "NoOp",
                                    "sync_info": si,
                                }
                                if inst.get("debug") is not None:
                                    nop["debug"] = inst["debug"]
                                out.append(nop)
                            continue  # drop the redundant load
                        last_sig = sig
                    elif op not in ("Matmult", "NoOp"):
                        last_sig = None
                    out.append(inst)
                bb["instructions"] = out
        # --- pass 2: split multi-wait sync_info onto NoOps ------------
        ctr = 0
        for f in m.get("functions", []):
            for bb in f.get("blocks", []):
                out = []
                for inst in bb.get("instructions", []):
                    si = inst.get("sync_info")
                    if si:
                        waits = si.get("on_wait") or []
                        if len(waits) > 1:
                            for wt in waits[:-1]:
                                ctr += 1
                                nop = {
                                    "engine": inst["engine"],
                                    "ins": [],
                                    "outs": [],
                                    "name": f"I-npw{ctr}",
                                    "opcode": "NoOp",
                                    "sync_info": {"on_wait": [wt], "on_update": []},
                                }
                                if inst.get("debug") is not None:
                                    nop["debug"] = inst["debug"]
                                out.append(nop)
                            si["on_wait"] = waits[-1:]
                    out.append(inst)
                bb["instructions"] = out
        return json.dumps(m).encode()

    bass.Bass.to_json_bytes = to_json_bytes
    bass.Bass._split_waits_patched = True


def _mm_noload(eng, out, lhsT, rhs, start, stop):
    """InstMatmult with ldweights=False: reuses the stationary already
    in the PE array (loaded by the preceding self-loading matmul with
    the same lhsT). lhsT is still passed as an input so Tile tracks the
    dependency, but walrus skips the redundant LDWEIGHTS."""
    ifmap_ap = eng.lower_ap(rhs.opt({0}), opt=False)
    weights_ap = eng.lower_ap(lhsT.opt({0}), opt=False, for_matmul_weights=True)
    out_ap = eng.lower_ap(out)
    return eng.add_instruction(
        mybir.InstMatmult(
            name=eng.bass.get_next_instruction_name(),
            replication_resolution=0,
            replication_shift_amnt=0,
            replication_num_rows=0,
            start_tensor_calc=start,
            stop_tensor_calc=stop,
            ldweights=False,
            ins=[ifmap_ap, weights_ap],
            outs=[out_ap],
            perf_mode=None,
            is_transpose=None,
            ifmap_quant_offset=None,
            weights_quant_offset=None,
            bass_skip_group_check=False,
            tile_position=(0, 0),
            tile_size=(128, 128),
        )
    )


def build_bass():
    _patch_split_sync_waits()
    nc = bass.Bass("TRN2", target_bir_lowering=False, debug=False)
    f32 = mybir.dt.float32
    fp16 = mybir.dt.float16

    qt = nc.dram_tensor("qt", [B_LOC, C, T], fp16, kind="ExternalInput")
    wr = nc.dram_tensor("wr", [C, N], fp16, kind="ExternalInput")
    o = nc.dram_tensor("o", [B_LOC, P, T, WW], fp16, kind="ExternalOutput")

    with tile.TileContext(nc) as tc:
        with ExitStack() as ctx:
            consts = ctx.enter_context(tc.tile_pool(name="consts", bufs=1))
            qpool = ctx.enter_context(tc.tile_pool(name="qpool", bufs=2))
            opool = ctx.enter_context(tc.tile_pool(name="opool", bufs=2))
            psum = ctx.enter_context(tc.tile_pool(name="psum", bufs=4, space="PSUM"))

            wr_sb = [
                consts.tile([128, N], fp16, tag=f"wr{k}", name=f"wr{k}")
                for k in range(2)
            ]
            nc.sync.dma_start(wr_sb[0][:], wr.ap()[0:128, :])
            nc.scalar.dma_start(wr_sb[1][:], wr.ap()[128:256, :])

            def load_q(b, eng0, eng1):
                q_sb = [
                    qpool.tile([128, T], fp16, tag=f"q{k}", name=f"q{k}_{b}")
                    for k in range(2)
                ]
                eng0.dma_start(q_sb[0][:], qt.ap()[b, 0:128, :])
                eng1.dma_start(q_sb[1][:], qt.ap()[b, 128:256, :])
                # t split as (tp, tl); lhsT slices are [c, tp] (stride TL)
                return [q_sb[k][:].rearrange("c (p l) -> c l p", l=TL) for k in range(2)]

            def compute_tl(pieces, q_v, tl):
                """pieces: list of (pt, nbase, nw). For each contraction
                half k, the stationary q tile is loaded once (self-loading
                first matmul) and reused by every subsequent chunk across
                all pieces (ldweights=False)."""
                for k in range(2):
                    first = True
                    for pt, nbase, nw in pieces:
                        for n0 in range(0, nw, 512):
                            n1 = min(n0 + 512, nw)
                            if USE_LD and not first:
                                _mm_noload(
                                    nc.tensor,
                                    pt[:, n0:n1],
                                    q_v[k][:, tl, :],
                                    wr_sb[k][:, nbase + n0 : nbase + n1],
                                    start=(k == 0),
                                    stop=(k == 1),
                                )
                            else:
                                nc.tensor.matmul(
                                    pt[:, n0:n1],
                                    q_v[k][:, tl, :],
                                    wr_sb[k][:, nbase + n0 : nbase + n1],
                                    start=(k == 0),
                                    stop=(k == 1),
                                )
                            first = False

            def evict(eng, oh, tl, pt, np_):
                """PSUM f32 -> SBUF fp16 downcast copy (bias is added on
                the host). eng is nc.vector (DVE) or nc.scalar (Act)."""
                pv = pt[:, 0 : np_ * WW].rearrange(
                    "t (p w) -> t p w", w=WW
                )
                dst = oh[:, :, bass.ds(tl * WW, WW)]
                if eng is nc.scalar:
                    eng.copy(dst, pv[:])
                else:
                    eng.tensor_copy(dst, pv[:])

            def store(eng, oh, b, p0, np_, src_off=0):
                dst = (
                    o.ap()[b, p0 : p0 + np_, :, :]
                    .rearrange("p (t l) w -> t p (l w)", l=TL)
                )
                eng.dma_start(dst, oh[:, src_off : src_off + np_, :])

            # ---- batch 0: two h-outer sweeps so the store stream starts
            # at ~1/8 of the kernel instead of ~1/4 ----
            q_v = load_q(0, nc.sync, nc.scalar)
            for h in range(2):
                oh = opool.tile([128, 16, TL * WW], fp16, tag=f"oh{h}", name=f"oh{h}_0")
                for tl in range(TL):
                    pt = psum.tile([128, 1024], f32, tag="pt", name=f"pt_0{h}_{tl}")
                    compute_tl([(pt, 784 * h, 784)], q_v, tl)
                    evict(nc.vector if tl % 2 == 0 else nc.scalar, oh, tl, pt, 16)
                    if h == 0 and tl == 0:
                        q_v_next = load_q(1, nc.gpsimd, nc.gpsimd)
                store(nc.sync, oh, 0, 16 * h, 8)
                store(nc.scalar, oh, 0, 16 * h + 8, 8, src_off=8)
            q_v = q_v_next

            # ---- batches 1..2: h-inner sweeps (stationary reused across
            # the full 1568 moving columns) ----
            for b in range(1, 3):
                oh = [
                    opool.tile([128, 16, TL * WW], fp16, tag=f"oh{h}", name=f"oh{h}_{b}")
                    for h in range(2)
                ]
                for tl in range(TL):
                    # two 2-bank PSUM tiles per tl (ring of 4) so evictions
                    # have 2 tl of slack and never stall the PE
                    ptA = psum.tile([128, 1024], f32, tag="pt", name=f"ptA_{b}_{tl}")
                    ptB = psum.tile([128, 1024], f32, tag="pt", name=f"ptB_{b}_{tl}")
                    compute_tl([(ptA, 0, 784), (ptB, 784, 784)], q_v, tl)
                    evict(nc.vector, oh[0], tl, ptA, 16)
                    evict(nc.scalar, oh[1], tl, ptB, 16)
                    if tl == 0:
                        # prefetch next batch's q via the Pool SWDGE queue
                        q_v_next = load_q(b + 1, nc.gpsimd, nc.gpsimd)
                store(nc.sync, oh[0], b, 0, 8)
                store(nc.scalar, oh[0], b, 8, 8, src_off=8)
                store(nc.sync, oh[1], b, 16, 8)
                store(nc.scalar, oh[1], b, 24, 8, src_off=8)
                q_v = q_v_next

            # ---- batch 3: h0 sweep, then p-strips of decreasing size so
            # the final stores are small (short tail after the last matmul).
            # The 8-wide strip rides the idle SWDGE queue to keep both
            # HWDGE queues free for the oh0 drain. ----
            oh0 = opool.tile([128, 16, TL * WW], fp16, tag="oh0", name="oh0_3")
            for tl in range(TL):
                pt = psum.tile([128, 1024], f32, tag="pt", name=f"pt_3h0_{tl}")
                compute_tl([(pt, 0, 784)], q_v, tl)
                evict(nc.vector if tl % 2 == 0 else nc.scalar, oh0, tl, pt, 16)
            store(nc.sync, oh0, 3, 0, 8)
            store(nc.scalar, oh0, 3, 8, 8, src_off=8)
            for s, (p0, np_) in enumerate([(16, 8), (24, 4), (28, 4)]):
                ohs = opool.tile(
                    [128, np_, TL * WW], fp16, tag=f"ohS{np_}", name=f"ohS_{s}"
                )
                for tl in range(TL):
                    pt = psum.tile([128, 1024], f32, tag="pt", name=f"pt_3s{s}_{tl}")
                    compute_tl([(pt, p0 * WW, np_ * WW)], q_v, tl)
                    evict(nc.vector if tl % 2 == 0 else nc.scalar, ohs, tl, pt, np_)
                if s == 0:
                    store(nc.gpsimd, ohs, 3, p0, np_)
                else:
                    hn = np_ // 2
                    store(nc.sync, ohs, 3, p0, hn)
                    store(nc.scalar, ohs, 3, p0 + hn, hn, src_off=hn)
    return nc


_NC_CACHE = None


def _get_nc():
    global _NC_CACHE
    if _NC_CACHE is None:
        _NC_CACHE = build_bass()
    return _NC_CACHE


def prep_inputs(q, W, b):
    """Host-side layout prep: weight packing + activation transpose +
    fp16 cast."""
    qt = np.ascontiguousarray(
        np.asarray(q, dtype=np.float32).transpose(0, 2, 1)
    ).astype(np.float16)  # [B, C, T]
    wr = np.ascontiguousarray(
        np.asarray(W, dtype=np.float32).transpose(2, 1, 0).reshape(C, N)
    ).astype(np.float16)
    return qt, wr


def assemble_output(core_outs, b):
    """Concatenate per-core fp16 device outputs, upcast to f32 and add
    the bias (b is the [W, P] reference bias) in one fused pass."""
    dev = np.concatenate(core_outs, axis=0)  # [B, P, T, W] fp16
    bias = np.asarray(b, dtype=np.float32).T[None, :, None, :]  # [1,P,1,W]
    return np.add(dev, bias, dtype=np.float32)


def kernel(q, W, b):
    qt, wr = prep_inputs(q, W, b)
    nc = _get_nc()
    in_maps = [
        {
            "qt": qt[c * B_LOC : (c + 1) * B_LOC],
            "wr": wr,
        }
        for c in range(N_CORES)
    ]
    res = run_bass_kernel_spmd(nc, in_maps, core_ids=list(range(N_CORES)))
    return assemble_output(
        [res.results[c]["o"] for c in range(N_CORES)], b
    )
